# revision 11
# baseline (speedup 1.0000x reference)
"""H2O Llama streaming attention (sparse_attention) Trainium2 Bass kernel.

Shards heads tensor-parallel across 8 NeuronCores: core c owns kv head c and
q heads 4c..4c+3.  Full inputs in, full outputs out; all heavy compute on
device.  Matmul operands are pre-cast to bf16 on the host (part of shard
prep); all accumulation and exact-copy outputs stay f32.
"""
import sys

sys.path.insert(0, "/opt/trn_rl_repo")

import os

import numpy as np

import concourse.bacc as bacc
import concourse.mybir as mybir
import concourse.tile as tile
from concourse.bass_utils import run_bass_kernel_spmd

# problem shapes (hardcoded per contract)
B, T, PAST, HID = 2, 512, 4096, 4096
HQ, HKV, D = 32, 8, 128
HH_SIZE, RECENT = 512, 512
CACHE = HH_SIZE + RECENT
S = PAST + T                  # 4608
NC_ = 8                       # cores
HQC = HQ // NC_               # 4 q heads per core
BT = B * T                    # 1024
NKT = HID // 128              # 32 contraction tiles
NST = S // 128                # 36 s-tiles
NPT = PAST // 128             # 32 past s-tiles
RSQRT_D = float(1.0 / np.sqrt(D))
F32 = mybir.dt.float32
BF16 = mybir.dt.bfloat16

_NC_CACHE = {}


def _build():
    if "nc" in _NC_CACHE:
        return _NC_CACHE["nc"]
    nc = bacc.Bacc(
        "TRN2", target_bir_lowering=False, debug=False, num_devices=NC_,
        dynamic_dma_scratch_size=65536,
    )
    LEVEL = int(os.environ.get("H2O_DEBUG_LEVEL", "5"))

    # ---------------- dram I/O ----------------
    hT_d = nc.dram_tensor("hT", [HID, BT], BF16, kind="ExternalInput")
    wq_d = nc.dram_tensor("wq", [HID, HQC * D], BF16, kind="ExternalInput")
    wk_d = nc.dram_tensor("wk", [HID, D], BF16, kind="ExternalInput")
    wv_d = nc.dram_tensor("wv", [HID, D], BF16, kind="ExternalInput")
    wo_d = nc.dram_tensor("wo", [HID, HID // NC_], BF16, kind="ExternalInput")
    pkT_d = nc.dram_tensor("pkT", [B, D, PAST], BF16, kind="ExternalInput")
    pvw_d = nc.dram_tensor("pvw", [B, 128, NPT, D], BF16, kind="ExternalInput")
    pk_d = nc.dram_tensor("pk", [B, PAST, D], F32, kind="ExternalInput")
    pv_d = nc.dram_tensor("pv", [B, PAST, D], F32, kind="ExternalInput")
    hhp_d = nc.dram_tensor("hhp", [B, 128, NPT], F32, kind="ExternalInput")
    cosT_d = nc.dram_tensor("cosT", [D, BT], F32, kind="ExternalInput")
    sinT_d = nc.dram_tensor("sinT", [D, BT], F32, kind="ExternalInput")
    iota_d = nc.dram_tensor("iota16", [16, PAST // 16], F32, kind="ExternalInput")
    cmask_d = nc.dram_tensor("cmask", [128, 4, T], BF16, kind="ExternalInput")
    ident_d = nc.dram_tensor("ident", [128, 128], F32, kind="ExternalInput")

    out_d = nc.dram_tensor("out", [BT, HID // NC_], F32, kind="ExternalOutput")
    kk_d = nc.dram_tensor("k_kept", [B, CACHE, D], F32, kind="ExternalOutput")
    vk_d = nc.dram_tensor("v_kept", [B, CACHE, D], F32, kind="ExternalOutput")
    hh_d = nc.dram_tensor("hh_new", [B, CACHE], F32, kind="ExternalOutput")

    # internal dram for the ctx all-gather (one per batch)
    cc_in = [nc.dram_tensor(f"cc_in{b}", [HQC * D, T], BF16) for b in range(B)]
    cc_out = [
        nc.dram_tensor(f"cc_out{b}", [NC_, HQC * D, T], BF16, addr_space="Shared")
        for b in range(B)
    ]

    with tile.TileContext(nc) as tc:
        with (
            tc.tile_pool(name="const", bufs=1) as cpool,
            tc.tile_pool(name="persist", bufs=1) as pp,
        ):
            ident = cpool.tile([128, 128], F32)
            nc.sync.dma_start(ident[:], ident_d[:])
            iota16 = cpool.tile([16, PAST // 16], F32)
            nc.sync.dma_start(iota16[:], iota_d[:])
            cmask = cpool.tile([128, 4, T], BF16)
            nc.sync.dma_start(cmask[:], cmask_d[:])
            cosT = cpool.tile([D, BT], F32)
            nc.sync.dma_start(cosT[:], cosT_d[:])
            sinT = cpool.tile([D, BT], F32)
            nc.sync.dma_start(sinT[:], sinT_d[:])
            hhp = cpool.tile([128, B, NPT], F32)
            nc.sync.dma_start(hhp[:], hhp_d[:].rearrange("b p c -> p b c"))
            ones_col = cpool.tile([128, 1], BF16)
            nc.vector.memset(ones_col[:], 1.0)
            ones_row = cpool.tile([1, 128], F32)
            nc.vector.memset(ones_row[:], 1.0)

            # persistent activations (filled in phase A, used in phase B)
            qTb = [
                pp.tile([128, BT], BF16, tag=f"qTb{h}", name=f"qTb{h}")
                for h in range(HQC)
            ]
            kTnb = pp.tile([128, BT], BF16)           # new kT (rope, bf16)
            vnb = pp.tile([128, BT // 128, D], BF16)  # new v bf16, t-block major

            # ---------------- phase A: projections + rope ----------------
            with (
                tc.tile_pool(name="phA", bufs=1) as pa,
                tc.tile_pool(name="psA", bufs=2, space="PSUM") as psA,
            ):
                hTb = pa.tile([128, NKT, BT], BF16)
                nc.sync.dma_start(hTb[:], hT_d[:].rearrange("(k p) t -> p k t", p=128))

                wkb = pa.tile([128, NKT, D], BF16)
                nc.sync.dma_start(wkb[:], wk_d[:].rearrange("(k p) t -> p k t", p=128))
                wvb = pa.tile([128, NKT, D], BF16)
                nc.sync.dma_start(wvb[:], wv_d[:].rearrange("(k p) t -> p k t", p=128))

                def rope(dst_bf, src_f32, also_f32=None):
                    qs = pa.tile([128, BT], F32, tag="ropeqs", bufs=1)
                    nc.vector.tensor_copy(qs[0:64, :], src_f32[64:128, :])
                    nc.vector.tensor_copy(qs[64:128, :], src_f32[0:64, :])
                    t1 = pa.tile([128, BT], F32, tag="ropet1", bufs=1)
                    nc.vector.tensor_tensor(
                        out=t1[:], in0=src_f32[:], in1=cosT[:], op=mybir.AluOpType.mult
                    )
                    nc.vector.tensor_tensor(
                        out=qs[:], in0=qs[:], in1=sinT[:], op=mybir.AluOpType.mult
                    )
                    if also_f32 is not None:
                        nc.vector.tensor_tensor(
                            out=also_f32[:], in0=t1[:], in1=qs[:],
                            op=mybir.AluOpType.add,
                        )
                        nc.vector.tensor_copy(dst_bf[:], also_f32[:])
                    else:
                        nc.vector.tensor_tensor(
                            out=dst_bf[:], in0=t1[:], in1=qs[:],
                            op=mybir.AluOpType.add,
                        )

                # q projection -> qT [d, t] per head, then rope
                for hpair in range(2):
                    wqb = pa.tile([128, NKT, 2 * D], BF16, tag="wqb", bufs=1)
                    nc.sync.dma_start(
                        wqb[:],
                        wq_d[:, hpair * 2 * D : (hpair + 1) * 2 * D].rearrange(
                            "(k p) t -> p k t", p=128
                        ),
                    )
                    for hl in range(2):
                        h = hpair * 2 + hl
                        qf = pa.tile([128, BT], F32, tag="projf", bufs=2)
                        for half in range(2):
                            q_ps = psA.tile([128, 512], F32, tag="psproj")
                            for k in range(NKT):
                                nc.tensor.matmul(
                                    q_ps[:],
                                    wqb[:, k, hl * D : (hl + 1) * D],
                                    hTb[:, k, half * 512 : (half + 1) * 512],
                                    start=(k == 0),
                                    stop=(k == NKT - 1),
                                )
                            nc.vector.tensor_copy(
                                qf[:, half * 512 : (half + 1) * 512], q_ps[:]
                            )
                        rope(qTb[h][:], qf)

                # k projection -> kT new + rope (keep f32 for exact output)
                kf = pa.tile([128, BT], F32, tag="projf", bufs=2)
                for half in range(2):
                    k_ps = psA.tile([128, 512], F32, tag="psproj")
                    for k in range(NKT):
                        nc.tensor.matmul(
                            k_ps[:],
                            wkb[:, k, :],
                            hTb[:, k, half * 512 : (half + 1) * 512],
                            start=(k == 0),
                            stop=(k == NKT - 1),
                        )
                    nc.vector.tensor_copy(kf[:, half * 512 : (half + 1) * 512], k_ps[:])
                knf = pa.tile([128, BT], F32)
                rope(kTnb[:], kf, also_f32=knf)

                # transpose rope'd k_new back to [t, d] for the k_kept output
                for b in range(B):
                    kout = pa.tile([128, 4, 128], F32, tag="kout", bufs=1)
                    for j in range(4):
                        tp = psA.tile([128, 128], F32, tag="pstp")
                        nc.tensor.transpose(
                            tp[:],
                            knf[:, b * 512 + j * 128 : b * 512 + (j + 1) * 128],
                            ident[:],
                        )
                        nc.vector.tensor_copy(kout[:, j, :], tp[:])
                    nc.sync.dma_start(
                        kk_d[b, HH_SIZE:CACHE, :].rearrange("(j p) d -> p j d", p=128),
                        kout[:],
                    )

                # v projection [t, d]; f32 rows straight to v_kept output
                for tb in range(BT // 128):
                    v_ps = psA.tile([128, D], F32, tag="psproj")
                    for k in range(NKT):
                        nc.tensor.matmul(
                            v_ps[:],
                            hTb[:, k, tb * 128 : (tb + 1) * 128],
                            wvb[:, k, :],
                            start=(k == 0),
                            stop=(k == NKT - 1),
                        )
                    vf = pa.tile([128, D], F32, tag="vf", bufs=3)
                    nc.vector.tensor_copy(vf[:], v_ps[:])
                    b, j = tb // 4, tb % 4
                    nc.sync.dma_start(
                        vk_d[b, HH_SIZE + j * 128 : HH_SIZE + (j + 1) * 128, :], vf[:]
                    )
                    nc.vector.tensor_copy(vnb[:, tb, :], vf[:])

            # ---------------- phase B: attention + H2O ----------------
            with (
                tc.tile_pool(name="phB", bufs=1) as pb,
                tc.tile_pool(name="psB", bufs=2, space="PSUM") as psB,
            ):
                wob = pb.tile([128, NKT, 512], BF16)
                nc.sync.dma_start(wob[:], wo_d[:].rearrange("(k p) t -> p k t", p=128))
                kTb = [
                    pb.tile([128, PAST], BF16, tag=f"kTb{b}", name=f"kTb{b}")
                    for b in range(B)
                ]
                vb = [
                    pb.tile([128, NPT, D], BF16, tag=f"vb{b}", name=f"vb{b}")
                    for b in range(B)
                ]
                for b in range(B):
                    nc.sync.dma_start(kTb[b][:], pkT_d[b])
                    nc.sync.dma_start(vb[b][:], pvw_d[b])

                sca = pb.tile([128, B * HQC * NST], F32)  # per-pair column sums
                for b in range(B if LEVEL >= 2 else 0):
                    for h in range(HQC):
                        pi = b * HQC + h
                        expts = []
                        r_ps = psB.tile([1, 512], F32, tag="rbc")
                        qslice = qTb[h][:, b * 512 : (b + 1) * 512]
                        for st in range(NST):
                            if st % 18 == 0:
                                ex = pb.tile(
                                    [128, 18, 512], BF16, tag="expt", bufs=2,
                                    name=f"expt{pi}_{st // 18}",
                                )
                                expts.append(ex)
                            if st < NPT:
                                ksl = kTb[b][:, st * 128 : (st + 1) * 128]
                            else:
                                j = st - NPT
                                ksl = kTnb[
                                    :, b * 512 + j * 128 : b * 512 + (j + 1) * 128
                                ]
                            sc_ps = psB.tile([128, 512], F32, tag="scps", bufs=3)
                            nc.tensor.matmul(
                                sc_ps[:], ksl, qslice, start=True, stop=True
                            )
                            esl = expts[st // 18][:, st % 18, :]
                            nc.scalar.activation(
                                esl, sc_ps[:], mybir.ActivationFunctionType.Exp,
                                scale=RSQRT_D,
                            )
                            if st >= NPT:
                                nc.vector.tensor_tensor(
                                    out=esl, in0=esl, in1=cmask[:, st - NPT, :],
                                    op=mybir.AluOpType.mult,
                                )
                            nc.tensor.matmul(
                                r_ps[:], ones_col[:], esl,
                                start=(st == 0), stop=(st == NST - 1),
                            )
                        rr = pb.tile([1, 512], F32, tag="rr", bufs=2)
                        nc.vector.reciprocal(rr[:], r_ps[:])
                        bc_ps = psB.tile([128, 512], F32, tag="rbc")
                        nc.tensor.matmul(
                            bc_ps[:], ones_row[:], rr[:], start=True, stop=True
                        )
                        rb = pb.tile([128, 512], BF16, tag="rb", bufs=2)
                        nc.vector.tensor_copy(rb[:], bc_ps[:])

                        ctx_ps = psB.tile([128, 512], F32, tag="ctxps")
                        for st in range(NST):
                            if st < NPT:
                                vsl = vb[b][:, st, :]
                            else:
                                vsl = vnb[:, b * 4 + (st - NPT), :]
                            attn = pb.tile([128, 512], BF16, tag="attn", bufs=2)
                            nc.vector.scalar_tensor_tensor(
                                attn[:], expts[st // 18][:, st % 18, :], 0.0, rb[:],
                                mybir.AluOpType.add, mybir.AluOpType.mult,
                                accum_out=sca[:, pi * NST + st : pi * NST + st + 1],
                            )
                            nc.tensor.matmul(
                                ctx_ps[:], vsl, attn[:],
                                start=(st == 0), stop=(st == NST - 1),
                            )
                        ctxb = pb.tile([128, 512], BF16, tag="ctxb", bufs=2)
                        nc.vector.tensor_copy(ctxb[:], ctx_ps[:])
                        nc.sync.dma_start(
                            cc_in[b][h * 128 : (h + 1) * 128, :], ctxb[:]
                        )

                    # ---- per-batch H2O eviction ----
                    ssum = pb.tile([128, NST], F32, tag="ssum", bufs=2)
                    nc.vector.tensor_tensor(
                        out=ssum[:],
                        in0=sca[:, (b * HQC + 0) * NST : (b * HQC + 1) * NST],
                        in1=sca[:, (b * HQC + 1) * NST : (b * HQC + 2) * NST],
                        op=mybir.AluOpType.add,
                    )
                    ssum2 = pb.tile([128, NST], F32, tag="ssum2", bufs=2)
                    nc.vector.tensor_tensor(
                        out=ssum2[:],
                        in0=sca[:, (b * HQC + 2) * NST : (b * HQC + 3) * NST],
                        in1=sca[:, (b * HQC + 3) * NST : (b * HQC + 4) * NST],
                        op=mybir.AluOpType.add,
                    )
                    nc.vector.tensor_tensor(
                        out=ssum[:], in0=ssum[:], in1=ssum2[:], op=mybir.AluOpType.add
                    )
                    # hh (past) = 0.25 * ssum[:, :32] + hh_prev
                    hhsel = pb.tile([128, NPT], F32, tag="hhsel", bufs=2)
                    nc.vector.scalar_tensor_tensor(
                        hhsel[:], ssum[:, 0:NPT], 0.25, hhp[:, b, :],
                        mybir.AluOpType.mult, mybir.AluOpType.add,
                    )
                    # hh (recent window) = 0.25 * ssum[:, 32:36] -> output
                    hhrec = pb.tile([128, 4], F32, tag="hhrec", bufs=2)
                    nc.vector.tensor_scalar(
                        hhrec[:], ssum[:, NPT:NST], 0.25, None, mybir.AluOpType.mult
                    )
                    nc.sync.dma_start(
                        hh_d[b, HH_SIZE:CACHE].rearrange("(c p) -> p c", p=128),
                        hhrec[:],
                    )
                    # threshold = 512th largest of the 4096 past hh values
                    if LEVEL < 3:
                        continue
                    th = pb.tile([1, 2], F32, tag="th", bufs=2)
                    qq = 1.0 - (HH_SIZE - 1.5) / (PAST - 1)
                    nc.gpsimd.kth_largest(
                        th[:], hhsel[:], n_per_lane=NPT, k=HH_SIZE - 2, quantile=qq
                    )
                    thb_ps = psB.tile([16, 1], F32, tag="rbc")
                    nc.tensor.matmul(
                        thb_ps[:], ones_row[:, 0:16], th[0:1, 1:2],
                        start=True, stop=True,
                    )
                    thb = pb.tile([16, 1], F32, tag="thb", bufs=2)
                    nc.vector.tensor_copy(thb[:], thb_ps[:])
                    # relayout hh_sel [128, 32] -> [16, 256] (16-wrapped)
                    hh16 = pb.tile([16, PAST // 16], F32, tag="hh16", bufs=2)
                    hh16v = hh16[:].rearrange("p (c g) -> p c g", g=8)
                    for pg in range(8):
                        nc.sync.dma_start(
                            hh16v[:, :, pg], hhsel[pg * 16 : (pg + 1) * 16, :]
                        )
                    mask16 = pb.tile(
                        [16, PAST // 16], mybir.dt.uint8, tag="m16", bufs=2
                    )
                    nc.vector.tensor_scalar(
                        mask16[:], hh16[:], thb[:], None, mybir.AluOpType.is_ge
                    )
                    selidx = pb.tile([16, PAST // 16], F32, tag="selidx", bufs=2)
                    selval = pb.tile([16, PAST // 16], F32, tag="selval", bufs=2)
                    nc.vector.memset(selidx[:], -1.0)
                    nc.vector.memset(selval[:], -1.0)
                    nc.vector.copy_predicated(selidx[:], mask16[:], iota16[:])
                    nc.vector.copy_predicated(selval[:], mask16[:], hh16[:])
                    idx16 = pb.tile([16, HH_SIZE // 16], F32, tag="idx16", bufs=2)
                    val16 = pb.tile([16, HH_SIZE // 16], F32, tag="val16", bufs=2)
                    nf = pb.tile([1, 1], mybir.dt.uint32, tag="nf", bufs=2)
                    nf2 = pb.tile([1, 1], mybir.dt.uint32, tag="nf2", bufs=2)
                    nc.gpsimd.sparse_gather(idx16[:], selidx[:], num_found=nf[:])
                    nc.gpsimd.sparse_gather(val16[:], selval[:], num_found=nf2[:])
                    nc.sync.dma_start(
                        hh_d[b, 0:HH_SIZE].rearrange("(c p) -> p c", p=16), val16[:]
                    )
                    idxi = pb.tile(
                        [16, HH_SIZE // 16], mybir.dt.int16, tag="idxi", bufs=2
                    )
                    nc.vector.tensor_copy(idxi[:], idx16[:])
                    idxr = pb.tile(
                        [128, HH_SIZE // 16], mybir.dt.int16, tag="idxr", bufs=2
                    )
                    nc.vector.tensor_copy(idxr[0:16, :], idxi[:])
                    for rep in range(1, 8):
                        nc.sync.dma_start(
                            idxr[rep * 16 : (rep + 1) * 16, :], idxr[0:16, :]
                        )
                    gk = pb.tile([128, HH_SIZE // 128, D], F32, tag="gk", bufs=2)
                    nc.gpsimd.dma_gather(
                        gk[:], pk_d[b], idxr[:], num_idxs=HH_SIZE,
                        num_idxs_reg=HH_SIZE, elem_size=D,
                    )
                    nc.sync.dma_start(
                        kk_d[b, 0:HH_SIZE, :].rearrange("(c p) d -> p c d", p=128),
                        gk[:],
                    )
                    gv = pb.tile([128, HH_SIZE // 128, D], F32, tag="gv", bufs=2)
                    nc.gpsimd.dma_gather(
                        gv[:], pv_d[b], idxr[:], num_idxs=HH_SIZE,
                        num_idxs_reg=HH_SIZE, elem_size=D,
                    )
                    nc.sync.dma_start(
                        vk_d[b, 0:HH_SIZE, :].rearrange("(c p) d -> p c d", p=128),
                        gv[:],
                    )

                    # ---- all-gather ctx for this batch + out projection ----
                    if LEVEL < 4:
                        continue
                    nc.gpsimd.collective_compute(
                        "AllGather", mybir.AluOpType.bypass,
                        replica_groups=[list(range(NC_))],
                        ins=[cc_in[b][:]], outs=[cc_out[b][:]],
                    )
                    ccflat = cc_out[b][:].rearrange("c q t -> (c q) t")
                    if LEVEL < 5:
                        continue
                    for tb in range(4):
                        o_ps = psB.tile([128, 512], F32, tag="ctxps")
                        for k in range(NKT):
                            cg = pb.tile([128, 128], BF16, tag="ctxg", bufs=4)
                            nc.sync.dma_start(
                                cg[:],
                                ccflat[
                                    k * 128 : (k + 1) * 128,
                                    tb * 128 : (tb + 1) * 128,
                                ],
                            )
                            nc.tensor.matmul(
                                o_ps[:],
                                cg[:],
                                wob[:, k, :],
                                start=(k == 0),
                                stop=(k == NKT - 1),
                            )
                        of = pb.tile([128, 512], F32, tag="of", bufs=2)
                        nc.vector.tensor_copy(of[:], o_ps[:])
                        nc.sync.dma_start(
                            out_d[b * 512 + tb * 128 : b * 512 + (tb + 1) * 128, :],
                            of[:],
                        )

    nc.compile()
    _NC_CACHE["nc"] = nc
    return nc


def _host_shard(inputs):
    import ml_dtypes

    bf = ml_dtypes.bfloat16
    hidden = np.asarray(inputs["hidden_states"], np.float32)
    past_k = np.asarray(inputs["past_k"], np.float32)
    past_v = np.asarray(inputs["past_v"], np.float32)
    hh_prev = np.asarray(inputs["hh_prev"], np.float32)
    wq = np.asarray(inputs["wq"], np.float32)
    wk = np.asarray(inputs["wk"], np.float32)
    wv = np.asarray(inputs["wv"], np.float32)
    wo = np.asarray(inputs["wo"], np.float32)
    cos = np.asarray(inputs["cos"], np.float32)
    sin = np.asarray(inputs["sin"], np.float32)
    pos = np.asarray(inputs["position_ids"])

    hT = np.ascontiguousarray(hidden.reshape(BT, HID).T.astype(bf))
    cos_g = cos[0][pos]                        # [B, T, D]
    sin_g = sin[0][pos]
    cosT = np.ascontiguousarray(
        np.concatenate([cos_g[b].T for b in range(B)], axis=1)
    )  # [D, B*T] b-major columns
    sin_signed = sin_g.copy()
    sin_signed[:, :, : D // 2] *= -1.0
    sinT = np.ascontiguousarray(
        np.concatenate([sin_signed[b].T for b in range(B)], axis=1)
    )
    iota16 = np.ascontiguousarray(
        np.arange(PAST, dtype=np.float32).reshape(PAST // 16, 16).T
    )
    tloc = np.arange(T)
    cm = np.zeros((128, 4, T), np.float32)
    for j in range(4):
        p = np.arange(128)[:, None]
        cm[:, j, :] = (128 * j + p <= tloc[None, :]).astype(np.float32)
    cmask = cm.astype(bf)
    ident = np.eye(128, dtype=np.float32)
    wq_bf = wq.astype(bf)
    wk_bf = wk.astype(bf)
    wv_bf = wv.astype(bf)
    wo_bf = wo.astype(bf)

    in_maps = []
    for c in range(NC_):
        qsl = slice(c * HQC * D, (c + 1) * HQC * D)
        ksl = slice(c * D, (c + 1) * D)
        osl = slice(c * (HID // NC_), (c + 1) * (HID // NC_))
        pvw = np.ascontiguousarray(
            past_v[:, c].reshape(B, NPT, 128, D).transpose(0, 2, 1, 3).astype(bf)
        )
        in_maps.append(
            {
                "hT": hT,
                "wq": np.ascontiguousarray(wq_bf[:, qsl]),
                "wk": np.ascontiguousarray(wk_bf[:, ksl]),
                "wv": np.ascontiguousarray(wv_bf[:, ksl]),
                "wo": np.ascontiguousarray(wo_bf[:, osl]),
                "pkT": np.ascontiguousarray(
                    past_k[:, c].transpose(0, 2, 1).astype(bf)
                ),
                "pvw": pvw,
                "pk": np.ascontiguousarray(past_k[:, c]),
                "pv": np.ascontiguousarray(past_v[:, c]),
                "hhp": np.ascontiguousarray(
                    hh_prev[:, c].reshape(B, NPT, 128).transpose(0, 2, 1)
                ),
                "cosT": cosT,
                "sinT": sinT,
                "iota16": iota16,
                "cmask": cmask,
                "ident": ident,
            }
        )
    return in_maps


def kernel(**inputs):
    nc = _build()
    in_maps = _host_shard(inputs)
    res = run_bass_kernel_spmd(nc, in_maps, list(range(NC_)))
    outs = res.results
    out = np.concatenate([outs[c]["out"] for c in range(NC_)], axis=1)
    out = out.reshape(B, T, HID)
    k_kept = np.stack([outs[c]["k_kept"] for c in range(NC_)], axis=1)
    v_kept = np.stack([outs[c]["v_kept"] for c in range(NC_)], axis=1)
    hh_new = np.stack([outs[c]["hh_new"] for c in range(NC_)], axis=1)
    return out, k_kept, v_kept, hh_new
